# revision 27
# baseline (speedup 1.0000x reference)
"""Distributed MHA kernel for 8 Trainium2 NeuronCores.

Sharding: core i handles batch b = i//2, head-group g = i%2 (8 of 16 heads).
Data parallel on B, tensor parallel on H: column-parallel QKV, row-parallel
output projection with the partial sums reduced on the host during gather.

Math (per core, heads h in its group, E=1024, H=16, d=64, N=1024):
  QT[hd, n] = sum_e Wq[hd, e] x[n, e] + bq[hd]        (transposed layout)
  KT[hd, n] = likewise
  V[n, hd]  = sum_e x[n, e] Wv[hd, e]                  (bv folded on host)
  energyT_h[k, q] = sum_d KT_h[d, k] QT_h[d, q]
  expT_h = exp(energyT_h)          (no max-subtract; |energy| < ~50 is safe)
  outT_h_aug[0:64, q] = sum_k V_h[k, d] expT_h[k, q]   (ones column in V
  outT_h_aug[64, q]   = sum_k expT_h[k, q] = den_h[q]   gives den for free)
  norm_h[d, q] = outT_h[d, q] * (1/32) * (1/den_h[q])
  y_part[q, e] = sum_{h,d} norm_h[d, q] Wo[e, 64h+d]
Host: out[b] = y_part[2b] + y_part[2b+1] + (bo + Wo @ bv / 32)
(att rows sum to 1/sqrt(E)=1/32 exactly, so bv contributes Wo@bv/32.)

Perf notes: all matmul operands 16-bit (fp32r streams at ~2 cyc/row on HW;
fp16/bf16 at 1 — warm issue gap 216 ns measured). fp16 for the Q/K path
(bf16's 8-bit mantissa noise is amplified through exp; diagonal energies
have std ~32); bf16 only for exp tiles (range: energies reach ~70) and V
(must match exp dtype in the att@V matmul). 1/(den) = exp(-ln(den*2^-64))
on ScalarE (DVE reciprocal is 8 cyc/elem on one lane; the custom fast-recip
op fails walrus codegen), broadcast across partitions via a DRAM bounce.
Stage-A/C matmul groups plus reader-free dummy matmuls are interleaved into
the ACT-bound attention loop as PE filler: the HAM clock gate halves the PE
clock after any >3.4us idle and only rearms after 3.4us of sustained work,
so PE duty must stay high everywhere. Dummies cycle the "ps" PSUM tag (its
slots free after quick DVE evacs), never "pen" (would inherit exp waits).
"""

import numpy as np
import ml_dtypes

import concourse.bass as bass
import concourse.tile as tile
from concourse import mybir
from concourse.bass_utils import run_bass_kernel_spmd

E = 1024
N = 1024
B = 4
NC = 8
EH = 512          # head dims per core (8 heads x 64)
D = 64
BF16 = mybir.dt.bfloat16
FP16 = mybir.dt.float16
F32 = mybir.dt.float32
AX = mybir.AluOpType


def split_drain_waits(nc):
    """Walrus in this toolchain rejects instructions carrying more than one
    sem wait; move extra waits onto injected same-engine NOPs placed right
    before the instruction (same engine queue = program order preserved)."""
    def take_nop(engine):
        nop = nc.engines[engine].nop(nofuse=True).ins
        for bname, bw in nc.bb_map.items():
            lst = bw.bb.instructions
            if lst and lst[-1].name == nop.name:
                bw.bb.instructions = lst[:-1]
                break
        return nop

    for name, w in list(nc.bb_map.items()):
        bb = w.bb
        # snapshot: take_nop mutates the live list (append+strip); iterating
        # it directly would re-visit the injected nops at the block tail
        new_insts = []
        changed = False
        for ins in list(bb.instructions):
            si = ins.sync_info
            if si is not None and si.on_wait and len(si.on_wait) > 1:
                waits = list(si.on_wait)
                for wt in waits[:-1]:
                    nop = take_nop(ins.engine)
                    nop.sync_info = mybir.SyncInfo(on_wait=[wt], on_update=[])
                    new_insts.append(nop)
                si.on_wait = waits[-1:]
                ins.sync_info = si
                changed = True
            new_insts.append(ins)
        if changed:
            bb.instructions = new_insts


def _emit(nc: bass.Bass, tc: tile.TileContext, ctx):
    xTa = nc.declare_dram_parameter("xTa", [128, 8, 512], FP16, isOutput=False)
    xTb = nc.declare_dram_parameter("xTb", [128, 8, 512], FP16, isOutput=False)
    wqT = nc.declare_dram_parameter("wqT", [128, 8, EH], FP16, isOutput=False)
    wkT = nc.declare_dram_parameter("wkT", [128, 8, EH], FP16, isOutput=False)
    wvT = nc.declare_dram_parameter("wvT", [128, 8, EH], FP16, isOutput=False)
    woT = nc.declare_dram_parameter("woT", [128, 4, E], FP16, isOutput=False)
    bqd = nc.declare_dram_parameter("bq", [128, 4, 1], F32, isOutput=False)
    bkd = nc.declare_dram_parameter("bk", [128, 4, 1], F32, isOutput=False)
    y = nc.declare_dram_parameter("y", [N, E], FP16, isOutput=True)

    persist = ctx.enter_context(tc.tile_pool(name="persist", bufs=1))
    work = ctx.enter_context(tc.tile_pool(name="work", bufs=2))
    et_pool = ctx.enter_context(tc.tile_pool(name="etp", bufs=4))
    ytr = ctx.enter_context(tc.tile_pool(name="ytr", bufs=2))
    dram = ctx.enter_context(tc.tile_pool(name="dram", bufs=4, space="DRAM"))
    psum2 = ctx.enter_context(tc.tile_pool(name="psum2", bufs=2, space="PSUM"))

    # ---- input loads, in first-use order (one big DMA per tensor) ----
    wq = persist.tile([128, 8, EH], FP16, tag="wq", name="wq")
    nc.sync.dma_start(out=wq, in_=wqT[:, :, :])
    bq_sb = persist.tile([128, 4, 1], F32, tag="bq", name="bq")
    nc.sync.dma_start(out=bq_sb, in_=bqd[:, :, :])
    xta = persist.tile([128, 8, 512], FP16, tag="xta", name="xta")
    nc.sync.dma_start(out=xta, in_=xTa[:, :, :])
    xtb = persist.tile([128, 8, 512], FP16, tag="xtb", name="xtb")
    nc.sync.dma_start(out=xtb, in_=xTb[:, :, :])
    wk = persist.tile([128, 8, EH], FP16, tag="wk", name="wk")
    nc.sync.dma_start(out=wk, in_=wkT[:, :, :])
    bk_sb = persist.tile([128, 4, 1], F32, tag="bk", name="bk")
    nc.sync.dma_start(out=bk_sb, in_=bkd[:, :, :])
    wv = persist.tile([128, 8, EH], FP16, tag="wv", name="wv")
    nc.sync.dma_start(out=wv, in_=wvT[:, :, :])
    wo = persist.tile([128, 4, E], FP16, tag="wo", name="wo")
    nc.sync.dma_start(out=wo, in_=woT[:, :, :])

    def xsl(e, lo, sz):
        """rhs/lhsT slice of x: [128 epart, n-cols lo:lo+sz] of chunk e."""
        if lo >= 512:
            return xtb[:, e, lo - 512:lo - 512 + sz]
        return xta[:, e, lo:lo + sz]

    # ---- PE warmup: dummy matmuls on a memset tile while DMAs land ----
    warm = persist.tile([128, 512], FP16, tag="warm", name="warm")
    nc.gpsimd.memset(warm, 0.0)
    ones_sb = persist.tile([65, 64], BF16, tag="ones", name="ones")
    nc.gpsimd.memset(ones_sb, 1.0)
    wps = psum2.tile([128, 512], F32, tag="ps", name="wps")
    for i in range(104):
        nc.tensor.matmul(out=wps[:, 0:128], lhsT=warm[:, 0:128], rhs=warm[:, 0:128],
                         start=(i == 0), stop=(i == 103))

    # ---- persistent stage outputs ----
    qt = [persist.tile([128, N], FP16, tag=f"qt{m}", name=f"qt{m}")
          for m in range(4)]
    kt = [persist.tile([128, N], FP16, tag=f"kt{m}", name=f"kt{m}")
          for m in range(4)]
    vt = [persist.tile([128, 8, 65], BF16, tag=f"v{n}", name=f"v{n}")
          for n in range(8)]
    pack = [[persist.tile([128, 512], FP16, tag=f"pk{p}_{qs}",
                          name=f"pk{p}_{qs}") for qs in range(2)]
            for p in range(4)]

    # ---- stage A groups (each: 8 accum MMs + 1 evac) ----
    def emit_qk_half(w_sb, b_sb, dst, m, half):
        ps = psum2.tile([128, 512], F32, tag="ps", name="psA")
        for e in range(8):
            nc.tensor.matmul(
                out=ps, lhsT=w_sb[:, e, m * 128:(m + 1) * 128],
                rhs=xsl(e, half * 512, 512), start=(e == 0), stop=(e == 7))
        nc.vector.tensor_scalar_add(
            dst[:, half * 512:(half + 1) * 512], ps, b_sb[:, m, :])

    def emit_v(n):
        ps = psum2.tile([128, 512], F32, tag="ps", name="psV")
        for e in range(8):
            nc.tensor.matmul(
                out=ps, lhsT=xsl(e, n * 128, 128), rhs=wv[:, e, :],
                start=(e == 0), stop=(e == 7))
        nc.vector.memset(vt[n][:, :, 64:65], 1.0)
        nc.vector.tensor_copy(
            vt[n][:, :, 0:64], ps.rearrange("p (h d) -> p h d", h=8))

    def emit_c_group(qs, qq, es):
        """One stage-C output tile [128 q, 512 e]: accumulate over 4 pairs."""
        ps = psum2.tile([128, 512], F32, tag="ps", name="psC")
        for p in range(4):
            nc.tensor.matmul(
                out=ps, lhsT=pack[p][qs][:, qq * 128:(qq + 1) * 128],
                rhs=wo[:, p, es * 512:(es + 1) * 512],
                start=(p == 0), stop=(p == 3))
        ys = ytr.tile([128, 512], FP16, tag="ysb", name="ysb")
        nc.vector.tensor_copy(ys, ps)
        qi = qs * 4 + qq
        nc.sync.dma_start(
            out=y[qi * 128:(qi + 1) * 128, es * 512:(es + 1) * 512], in_=ys)

    # filler generator: stage A work for pairs 1..3
    def filler_gen():
        for p in range(1, 4):
            for half in range(2):
                yield lambda p=p, half=half: emit_qk_half(wq, bq_sb, qt[p], p, half)
            for half in range(2):
                yield lambda p=p, half=half: emit_qk_half(wk, bk_sb, kt[p], p, half)

    fill = filler_gen()

    def pull_filler(k):
        try:
            next(fill)()
        except StopIteration:
            pass

    # ---- stage B: one attention unit per (head pair p, q-slice qs) ----
    def emit_b_unit(p, qs, per_k_fill):
        po = [psum2.tile([65, 512], F32, tag=f"po{ab}", name=f"po{ab}")
              for ab in range(2)]
        for k in range(8):
            ets = []
            for ab in range(2):
                pen = psum2.tile([128, 512], F32, tag="pen", name="pen")
                nc.tensor.matmul(
                    out=pen,
                    lhsT=kt[p][ab * 64:(ab + 1) * 64, k * 128:(k + 1) * 128],
                    rhs=qt[p][ab * 64:(ab + 1) * 64, qs * 512:(qs + 1) * 512],
                    start=True, stop=True)
                et = et_pool.tile([128, 512], BF16, tag="et", name="et")
                nc.scalar.activation(
                    out=et, in_=pen, func=mybir.ActivationFunctionType.Exp)
                ets.append(et)
            for ab in range(2):
                nc.tensor.matmul(
                    out=po[ab], lhsT=vt[k][:, 2 * p + ab, :], rhs=ets[ab],
                    start=(k == 0), stop=(k == 7))
            per_k_fill(k)
        # normalization: 1/den = exp(-ln(den)) on ScalarE (recip is broken
        # on DVE-custom in this toolchain; Ln+Exp share one ACT table set).
        # Broadcast across partitions with a 1-row matmul (ones^T @ s2) --
        # far shorter than the DRAM-bounce DMA round trip. ln stays f32 (its
        # error is amplified by exp); the recip itself is fine in bf16.
        for ab in range(2):
            sr = work.tile([128, 512], F32, tag="sr", name="sr")
            nc.scalar.activation(
                out=sr[64:65, :], in_=po[ab][64:65, :],
                func=mybir.ActivationFunctionType.Ln, scale=2.0 ** -64)
            s2 = work.tile([128, 512], BF16, tag="s2", name="s2")
            nc.scalar.activation(
                out=s2[64:65, :], in_=sr[64:65, :],
                func=mybir.ActivationFunctionType.Exp, scale=-1.0)
            srep_ps = psum2.tile([64, 512], F32, tag="ps", name="srep_ps")
            nc.tensor.matmul(out=srep_ps, lhsT=ones_sb[64:65, :],
                             rhs=s2[64:65, :], start=True, stop=True)
            srep = work.tile([64, 512], F32, tag="srep", name="srep")
            nc.vector.tensor_copy(srep, srep_ps)
            if ab == 0:
                nc.vector.scalar_tensor_tensor(
                    out=pack[p][qs][0:64, :], in0=po[ab][0:64, :],
                    scalar=2.0 ** -69, in1=srep, op0=AX.mult, op1=AX.mult)
            else:
                tmp = work.tile([64, 512], FP16, tag="tmp", name="tmp")
                nc.vector.scalar_tensor_tensor(
                    out=tmp, in0=po[ab][0:64, :],
                    scalar=2.0 ** -69, in1=srep, op0=AX.mult, op1=AX.mult)
                nc.sync.dma_start(out=pack[p][qs][64:128, :], in_=tmp)

    # ---- emission schedule ----
    for half in range(2):
        emit_qk_half(wq, bq_sb, qt[0], 0, half)
    for half in range(2):
        emit_qk_half(wk, bk_sb, kt[0], 0, half)
    emit_v(0)
    emit_v(1)

    vq = list(range(2, 8))  # remaining V tiles, fed as early filler

    def vfill(k):
        if vq:
            emit_v(vq.pop(0))

    def emit_dummies(n):
        """Disposable PE filler: matmuls through the pen slot rotation with
        no readers. Keeps PE duty high in ACT-bound stretches so the HAM
        clock gate never drops to K=4/8 (a >3.4us idle would halve the PE
        clock until 3.4us of *sustained* busy, which an ACT-bound loop
        never reaches)."""
        for _ in range(n):
            dm = psum2.tile([128, 512], F32, tag="ps", name="dmy")
            nc.tensor.matmul(out=dm, lhsT=warm[:, 0:128], rhs=warm,
                             start=True, stop=True)

    # per-unit PE filler pacing: V tiles early, then stage-A for later pairs
    # (A(p) must be fully emitted before unit (p, 0)), C0 inside the last
    # unit, dummies anywhere real filler runs short of the ACT deficit
    for p in range(4):
        for qs in range(2):
            u = 2 * p + qs
            if u >= 4:
                # boundary insurance: ride through the previous unit's
                # den-normalize chain without a >3.4us PE idle
                emit_dummies(4)
            if u == 0:
                # feed V(2..7) one per k-chunk so vt[k] is ready in time
                emit_b_unit(p, qs, vfill)
            elif u == 1:
                emit_b_unit(p, qs, lambda k: pull_filler(k) if k % 2 else None)
            elif u in (2, 3, 4, 5):
                emit_b_unit(p, qs, lambda k: pull_filler(k) if k in (3, 7)
                            else emit_dummies(1))
            elif u == 6:
                emit_b_unit(p, qs, lambda k: emit_dummies(3))
            else:
                # stage C qs=0 groups as filler (pack[*][0] complete by now)
                emit_b_unit(p, qs, lambda k: (emit_c_group(0, k // 2, k % 2),
                                              emit_dummies(1)))
    # drain any unemitted stage-A filler
    for f in fill:
        f()
    # stage C qs=1 tail (dummies bridge the last den-normalize chain)
    emit_dummies(6)
    for qq in range(4):
        for es in range(2):
            emit_c_group(1, qq, es)


def build():
    from contextlib import ExitStack
    nc = bass.Bass()
    with tile.TileContext(nc) as tc:
        with ExitStack() as ctx:
            _emit(nc, tc, ctx)
    split_drain_waits(nc)
    return nc


def make_in_maps(x, Wq, bq, Wk, bk, Wv, bv, Wo, bo):
    bf = np.float16
    in_maps = []
    for i in range(NC):
        b, g = i // 2, i % 2
        sl = slice(g * EH, (g + 1) * EH)
        xT = np.ascontiguousarray(x[b].T)          # [E, N] f32
        # [E, N] -> [128, 8, N] with partition = E % 128 layout (e-chunk dim 2nd)
        xTr = xT.reshape(8, 128, N).transpose(1, 0, 2)
        wqr = np.ascontiguousarray(Wq[sl, :].T).reshape(8, 128, EH).transpose(1, 0, 2)
        wkr = np.ascontiguousarray(Wk[sl, :].T).reshape(8, 128, EH).transpose(1, 0, 2)
        wvr = np.ascontiguousarray(Wv[sl, :].T).reshape(8, 128, EH).transpose(1, 0, 2)
        wor = np.ascontiguousarray(Wo[:, sl].T).reshape(4, 128, E).transpose(1, 0, 2)
        in_maps.append({
            "xTa": np.ascontiguousarray(xTr[:, :, 0:512]).astype(bf),
            "xTb": np.ascontiguousarray(xTr[:, :, 512:1024]).astype(bf),
            "wqT": np.ascontiguousarray(wqr).astype(bf),
            "wkT": np.ascontiguousarray(wkr).astype(bf),
            "wvT": np.ascontiguousarray(wvr).astype(bf),
            "woT": np.ascontiguousarray(wor).astype(bf),
            "bq": np.ascontiguousarray(
                bq[sl].reshape(4, 128, 1).transpose(1, 0, 2)).astype(np.float32),
            "bk": np.ascontiguousarray(
                bk[sl].reshape(4, 128, 1).transpose(1, 0, 2)).astype(np.float32),
        })
    return in_maps


def gather(results, Wv_b, Wo, bv, bo):
    host_bias = (bo + Wo @ bv / 32.0).astype(np.float32)
    out = np.empty((B, N, E), np.float32)
    for b in range(B):
        out[b] = (results[2 * b]["y"].astype(np.float32)
                  + results[2 * b + 1]["y"].astype(np.float32) + host_bias)
    return out


def kernel(x, Wq, bq, Wk, bk, Wv, bv, Wo, bo):
    x, Wq, bq, Wk, bk, Wv, bv, Wo, bo = [
        np.asarray(a, np.float32) for a in (x, Wq, bq, Wk, bk, Wv, bv, Wo, bo)]
    nc = build()
    in_maps = make_in_maps(x, Wq, bq, Wk, bk, Wv, bv, Wo, bo)
    res = run_bass_kernel_spmd(nc, in_maps, list(range(NC)))
    return gather(res.results, Wv, Wo, bv, bo)


if __name__ == "__main__":
    import reference
    inputs = {k: np.asarray(v) for k, v in reference.setup_inputs().items()}
    out = kernel(**inputs)
    exp = np.asarray(reference.reference(**inputs))
    rel = np.abs(out - exp).max() / np.abs(exp).max()
    print("Relative error:", rel)
